# revision 23
# baseline (speedup 1.0000x reference)
"""Trainium2 Bass kernel for a binarized CNN (BinaryNet-style).

Network (per sample, input 1x15x15):
  conv1 3x3 s2 (1->4)  + hardtanh   -> 4x7x7
  conv2 3x3 s2 (4->8)  + hardtanh   -> 8x3x3
  conv3 3x3 s2 (8->16) + hardtanh   -> 16x1x1
  fc1 16->8 + hardtanh, fc2 8->1
with sign() binarization of inputs and weights at every layer.

Lowering:
  * Every post-conv value is a small integer; hardtanh+binarize == sign(),
    and sign(int) == clamp(v,-1,1) (one fused min/max tensor_scalar).
  * Convs become dense matmuls against block-Toeplitz expanded weights
    (225x196, 196x72, 72x16), batch on the matmul free dim.
  * Input sign: (x>=0)-0.5 in {-.5,+.5} (one DVE op per K-chunk), with the
    2x folded into the conv1 weight rows.
  * Tail layers (conv3/fc1/fc2) run as 4 concurrent 32-col/row tiled
    matmuls (tile_position) so their sign ops are [128,128] not [32,512].
Data parallel across 8 NeuronCores, batch split evenly.
"""

import numpy as np
import ml_dtypes
from contextlib import ExitStack

import concourse.bass as bass
import concourse.tile as tile
import concourse.mybir as mybir
from concourse.vector_clock import ScopedClock
from concourse.bass_utils import run_bass_kernel_spmd

# ----------------------------------------------------------------------------
# Workaround: this walrus build accepts only one sync-wait command per
# instruction; Tile attaches several. Split extras onto same-engine NoOps.
# ----------------------------------------------------------------------------
_MAX_WAITS = 1


def _patched_drain_and_barrier(self, tick_clock, wait_clock):
    probe = self.nc.sync.nop()
    wait_clock.add_sem_waits(probe.ins, ScopedClock({None: tick_clock.global_clock}))
    waits = list(probe.ins.sync_info.on_wait)
    probe.ins.sync_info.on_wait = waits[:_MAX_WAITS]
    rest = waits[_MAX_WAITS:]
    while rest:
        nop = self.nc.sync.nop()
        nop.ins.sync_info = mybir.SyncInfo(on_wait=rest[:_MAX_WAITS], on_update=[])
        rest = rest[_MAX_WAITS:]
    self.nc.sync.drain()
    self.nc.all_engine_barrier(sem_only=True)
    assert self.sems is not None
    popped = self.nc._tile_sem_poison_stack.pop()
    assert popped is self._sem_poison
    self.nc.clear_and_free_semaphores(list(self.sems.allocated().values()))
    self.nc.all_engine_barrier(sem_only=True)


tile.TileContext._drain_and_barrier = _patched_drain_and_barrier


def _split_multi_waits(nc, max_waits=_MAX_WAITS):
    """Move extra semaphore waits onto preceding same-engine NoOps."""
    for fn in nc.m.functions:
        for blk in fn.blocks:
            old = list(blk.instructions)
            if not any(
                i.sync_info is not None and len(i.sync_info.on_wait) > max_waits
                for i in old
            ):
                continue
            new = []
            for ins in old:
                si = ins.sync_info
                if si is not None and len(si.on_wait) > max_waits:
                    waits = list(si.on_wait)
                    head, keep = waits[:-max_waits], waits[-max_waits:]
                    for i in range(0, len(head), max_waits):
                        new.append(mybir.InstNoOp(
                            name=f"{ins.name}-sw{i}",
                            engine=ins.engine,
                            bass_nofuse=True,
                            sync_info=mybir.SyncInfo(
                                on_wait=head[i:i + max_waits], on_update=[]
                            ),
                        ))
                    ins.sync_info = mybir.SyncInfo(
                        on_wait=keep, on_update=list(si.on_update)
                    )
                new.append(ins)
            try:
                blk.instructions[:] = new
            except TypeError:
                blk.instructions = new

# ----------------------------------------------------------------------------
# Problem constants
# ----------------------------------------------------------------------------
N_CORES = 8
B_TOTAL = 131072
BC = B_TOTAL // N_CORES  # 16384 samples per core
NB = 512                 # samples per compute tile (one PSUM bank)
GROUP = 2048             # samples per input DMA transfer
K1 = 225                 # conv1 contraction (15*15)
KPAD = 256               # x padded to 256 rows so both DMA chunks are 128-partition
M1 = 196                 # conv1 outputs (4ch * 49pos)
K1A, K1B = 128, 97       # conv1 K split
M1A, M1B = 128, 68       # conv1 M split
K2A, K2B = 128, 68       # conv2 K split (196 total)
M2 = 72                  # conv2 outputs (8ch * 9pos)
BF16 = mybir.dt.bfloat16
F32 = mybir.dt.float32

# engine per elementwise op: 'v'=DVE, 'a'=ACT
ENG = {
    "z1a": "v", "z1b": "a",
    "z2": "a", "z3": "a",
    "z4": "v", "z5": "v",
}

FP8 = mybir.dt.float8e4
M1P = 112                # DR-padded conv1 output half (98 -> 112, step%16==0)
M2P = 80                 # DR-padded conv2 output (72 -> 80)


def _sign_host(w):
    return np.sign(w.astype(np.float32))


def build_weights(w1, w2, w3, wfc1, wfc2):
    """Expand the conv weights into the dense matmul operands."""
    s1, s2, s3 = _sign_host(w1), _sign_host(w2), _sign_host(w3)
    sf1, sf2 = _sign_host(wfc1), _sign_host(wfc2)

    # conv1: [225, 196], x2 to absorb the {-.5,.5} input encoding
    big1 = np.zeros((K1, M1), np.float32)
    for ch in range(4):
        for i in range(7):
            for j in range(7):
                m = ch * 49 + i * 7 + j
                for di in range(3):
                    for dj in range(3):
                        big1[(2 * i + di) * 15 + (2 * j + dj), m] = s1[ch, 0, di, dj]
    big1 *= 2.0

    # conv2: [196, 72]
    big2 = np.zeros((196, M2), np.float32)
    for ch in range(8):
        for i in range(3):
            for j in range(3):
                m = ch * 9 + i * 3 + j
                for ci in range(4):
                    for di in range(3):
                        for dj in range(3):
                            big2[ci * 49 + (2 * i + di) * 7 + (2 * j + dj), m] = \
                                s2[ch, ci, di, dj]

    # conv3: [72, 32]; cols 16:32 zero so the PSUM pad rows are written as 0
    big3 = np.zeros((72, 32), np.float32)
    for ch in range(16):
        for ci in range(8):
            for r in range(3):
                for c in range(3):
                    big3[ci * 9 + r * 3 + c, ch] = s3[ch, ci, r, c]

    # fc1: [32, 32] replicated to [128, 32] for the 4 row/col-tiled matmuls
    f1 = np.zeros((32, 32), np.float32)
    f1[:16, :8] = sf1.T
    f1r = np.tile(f1, (4, 1))
    # fc2: [32, 32] col 0 only, replicated to [128, 32]
    f2 = np.zeros((32, 32), np.float32)
    f2[:8, 0] = sf2[0, :]
    f2r = np.tile(f2, (4, 1))

    bf = ml_dtypes.bfloat16
    return {
        "w1": big1.astype(bf), "w2": big2.astype(bf), "w3": big3.astype(bf),
        "wf1": f1r.astype(bf), "wf2": f2r.astype(bf),
    }


def _sign_int(nc, eng, out_ap, in_ap):
    """sign() for integer-valued input: clamp(v,-1,1) on DVE, LUT on ACT."""
    if eng == "a":
        nc.scalar.sign(out_ap, in_ap)
    else:
        nc.vector.tensor_scalar(out_ap, in_ap, 1.0, -1.0,
                                mybir.AluOpType.min, mybir.AluOpType.max)


def _sign_x(nc, eng, out_ap, in_ap):
    """(x>=0) - 0.5 -> {-0.5, +0.5}; the 2x lives in the conv1 weights."""
    assert eng == "v"
    nc.vector.tensor_scalar(out_ap, in_ap, 0.0, 0.5,
                            mybir.AluOpType.is_ge, mybir.AluOpType.subtract)


def build_bass(bc=BC, split_waits=True):
    """Build the per-core Bass program (identical on all cores)."""
    assert bc % GROUP == 0
    n_groups = bc // GROUP
    tiles_per_group = GROUP // NB

    nc = bass.Bass()
    xt = nc.dram_tensor("xt", (128, 2, bc), F32, kind="ExternalInput")
    w1 = nc.dram_tensor("w1", (K1, M1), BF16, kind="ExternalInput")
    w2 = nc.dram_tensor("w2", (196, M2), BF16, kind="ExternalInput")
    w3 = nc.dram_tensor("w3", (72, 32), BF16, kind="ExternalInput")
    wf1 = nc.dram_tensor("wf1", (128, 32), BF16, kind="ExternalInput")
    wf2 = nc.dram_tensor("wf2", (128, 32), BF16, kind="ExternalInput")
    y = nc.dram_tensor("y", (4, bc // 4), F32, kind="ExternalOutput")

    with tile.TileContext(nc) as tc:
        with ExitStack() as ctx:
            consts = ctx.enter_context(tc.tile_pool(name="consts", bufs=1))
            xin = ctx.enter_context(tc.tile_pool(name="xin", bufs=2))
            sx_p = ctx.enter_context(tc.tile_pool(name="sx", bufs=2))
            s1_p = ctx.enter_context(tc.tile_pool(name="s1", bufs=3))
            s2_p = ctx.enter_context(tc.tile_pool(name="s2", bufs=3))
            s3_p = ctx.enter_context(tc.tile_pool(name="s3", bufs=3))
            s4_p = ctx.enter_context(tc.tile_pool(name="s4", bufs=3))
            yout = ctx.enter_context(tc.tile_pool(name="yout", bufs=2))
            ps_big = ctx.enter_context(tc.tile_pool(name="ps_big", bufs=2, space="PSUM"))
            ps_mid = ctx.enter_context(tc.tile_pool(name="ps_mid", bufs=2, space="PSUM"))
            ps_tail = ctx.enter_context(tc.tile_pool(name="ps_tail", bufs=2, space="PSUM"))

            # --- load weights once ---
            w1hi = consts.tile([K1A, M1], BF16)
            w1lo = consts.tile([K1B, M1], BF16)
            nc.sync.dma_start(out=w1hi, in_=w1[0:K1A, :])
            nc.sync.dma_start(out=w1lo, in_=w1[K1A:K1, :])
            w2hi = consts.tile([K2A, M2], BF16)
            w2lo = consts.tile([K2B, M2], BF16)
            nc.sync.dma_start(out=w2hi, in_=w2[0:K2A, :])
            nc.sync.dma_start(out=w2lo, in_=w2[K2A:196, :])
            w3t = consts.tile([72, 32], BF16)
            nc.sync.dma_start(out=w3t, in_=w3[:, :])
            wf1t = consts.tile([128, 32], BF16)
            nc.sync.dma_start(out=wf1t, in_=wf1[:, :])
            wf2t = consts.tile([128, 32], BF16)
            nc.sync.dma_start(out=wf2t, in_=wf2[:, :])

            for g in range(n_groups):
                g0 = g * GROUP
                # SWDGE DMA casts fp32 -> bf16 on the fly. The first group is
                # loaded and signed per-tile to prime the pipeline quickly.
                xg = xin.tile([128, 2, GROUP], F32, tag="xg")
                n_chunks = tiles_per_group if g == 0 else 1
                cw = GROUP // n_chunks
                for ci in range(n_chunks):
                    cs = slice(ci * cw, (ci + 1) * cw)
                    gs = slice(g0 + ci * cw, g0 + (ci + 1) * cw)
                    nc.sync.dma_start(out=xg[:, :, cs], in_=xt[:, :, gs])
                # one sign op per tile: both K-chunks in a single [128, 1024]
                sxs = []
                for ti in range(tiles_per_group):
                    ts_ = slice(ti * NB, (ti + 1) * NB)
                    sx = sx_p.tile([128, 2, NB], BF16, name=f"sx_{ti}", tag=f"sx{ti % 2}")
                    _sign_x(nc, "v", sx, xg[:, :, ts_])
                    sxs.append(sx)
                ysb = yout.tile([128, GROUP // 4], F32, tag="ysb")

                # process tiles in pairs so each big LDWEIGHTS serves 2 matmuls
                for tp in range(tiles_per_group // 2):
                    sls = [slice((2 * tp + uu) * NB, (2 * tp + uu + 1) * NB)
                           for uu in range(2)]

                    p1as = [ps_big.tile([M1A, NB], F32, name=f"p1a_{u}", tag="p1a")
                            for u in range(2)]
                    p1bs = [ps_big.tile([M1B, NB], F32, name=f"p1b_{u}", tag="p1b")
                            for u in range(2)]
                    for u in range(2):
                        nc.tensor.matmul(p1as[u], w1hi[:, 0:M1A], sxs[2 * tp + u][0:K1A, 0, :],
                                         start=True, stop=False)
                    for u in range(2):
                        nc.tensor.matmul(p1as[u], w1lo[:, 0:M1A], sxs[2 * tp + u][0:K1B, 1, :],
                                         start=False, stop=True)
                    for u in range(2):
                        nc.tensor.matmul(p1bs[u], w1hi[:, M1A:M1], sxs[2 * tp + u][0:K1A, 0, :],
                                         start=True, stop=False)
                    for u in range(2):
                        nc.tensor.matmul(p1bs[u], w1lo[:, M1A:M1], sxs[2 * tp + u][0:K1B, 1, :],
                                         start=False, stop=True)

                    s1as, s1bs = [], []
                    for u in range(2):
                        s1a = s1_p.tile([M1A, NB], BF16, name=f"s1a_{u}", tag=f"s1a{u}")
                        s1b = s1_p.tile([M1B, NB], BF16, name=f"s1b_{u}", tag=f"s1b{u}")
                        _sign_int(nc, ENG["z1a"], s1a, p1as[u])
                        _sign_int(nc, ENG["z1b"], s1b, p1bs[u])
                        s1as.append(s1a)
                        s1bs.append(s1b)

                    p2s = [ps_mid.tile([M2, NB], F32, name=f"p2_{u}", tag="p2")
                           for u in range(2)]
                    for u in range(2):
                        nc.tensor.matmul(p2s[u], w2hi, s1as[u], start=True, stop=False)
                    for u in range(2):
                        nc.tensor.matmul(p2s[u], w2lo, s1bs[u], start=False, stop=True)

                    for u in range(2):
                        t = 2 * tp + u
                        s2t = s2_p.tile([M2, NB], BF16, tag="s2")
                        _sign_int(nc, ENG["z2"], s2t, p2s[u])

                        # --- tail: 4 concurrent 32-wide tiled matmuls each;
                        # all 3 stages share one PSUM bank ---
                        ptail = ps_tail.tile([128, 384], F32, tag="tail")
                        p3 = ptail[:, 0:128]
                        for r in range(4):
                            nc.tensor.matmul(
                                p3[32 * r:32 * r + 32, :], w3t,
                                s2t[:, 128 * r:128 * r + 128],
                                start=True, stop=True, tile_position=(0, 32 * r),
                            )
                        s3t = s3_p.tile([128, 128], BF16, tag="s3")
                        _sign_int(nc, ENG["z3"], s3t, p3)

                        p4 = ptail[:, 128:256]
                        for r in range(4):
                            nc.tensor.matmul(
                                p4[32 * r:32 * r + 32, :],
                                wf1t[32 * r:32 * r + 32, :],
                                s3t[32 * r:32 * r + 32, :],
                                start=True, stop=True, tile_position=(32 * r, 32 * r),
                            )
                        s4t = s4_p.tile([128, 128], BF16, tag="s4")
                        _sign_int(nc, ENG["z4"], s4t, p4)

                        p5 = ptail[:, 256:384]
                        for r in range(4):
                            nc.tensor.matmul(
                                p5[32 * r:32 * r + 32, :],
                                wf2t[32 * r:32 * r + 32, :],
                                s4t[32 * r:32 * r + 32, :],
                                start=True, stop=True, tile_position=(32 * r, 32 * r),
                            )
                        nc.vector.tensor_copy(ysb[:, t * 128:(t + 1) * 128], p5)

                # y rows live at partitions {0,32,64,96}
                nc.sync.dma_start(
                    out=y[0:4, g * (GROUP // 4):(g + 1) * (GROUP // 4)],
                    in_=ysb[0:128:32, :],
                )
    if split_waits:
        _split_multi_waits(nc)
    return nc


_NC_CACHE = {}


def _get_nc(bc):
    if bc not in _NC_CACHE:
        _NC_CACHE[bc] = build_bass(bc)
    return _NC_CACHE[bc]


def host_inputs(x, w1, w2, w3, wfc1, wfc2, n_cores=N_CORES):
    """Shard + lay out the inputs for each core.

    xt layout: [128 pairs, 2, bc] where xt[k, i, s] = x[s, feature 2k+i]
    (features 225..255 zero-padded) — the DoubleRow rhs layout.
    """
    b = x.shape[0]
    bc = b // n_cores
    ws = build_weights(w1, w2, w3, wfc1, wfc2)
    xf = np.asarray(x, np.float32).reshape(b, K1)
    in_maps = []
    for c in range(n_cores):
        xc = np.zeros((128, 2, bc), np.float32)
        xcf = xf[c * bc:(c + 1) * bc].T
        xc[:, 0, :] = xcf[0:128]
        xc[0:K1B, 1, :] = xcf[128:K1]
        in_maps.append({"xt": xc, **ws})
    return in_maps, bc


def unshard_y(results, bc):
    """[4, bc/4] per core -> [8*bc, 1]; sample = g*2048 + t*512 + r*128 + c."""
    outs = []
    for r in results:
        yb = np.asarray(r["y"], np.float32)          # [4, bc/4]
        n_groups = bc // GROUP
        a = yb.reshape(4, n_groups, 4, 128)           # [r, g, t, c]
        outs.append(a.transpose(1, 2, 0, 3).reshape(bc))
    return np.concatenate(outs).reshape(-1, 1)


def kernel(x, w1, w2, w3, wfc1, wfc2):
    in_maps, bc = host_inputs(x, w1, w2, w3, wfc1, wfc2)
    nc = _get_nc(bc)
    res = run_bass_kernel_spmd(nc, in_maps, core_ids=list(range(N_CORES)))
    return unshard_y(res.results, bc).astype(np.float32)


# revision 24
# speedup vs baseline: 1.1899x; 1.1899x over previous
"""Trainium2 Bass kernel for a binarized CNN (BinaryNet-style).

Network (per sample, input 1x15x15):
  conv1 3x3 s2 (1->4)  + hardtanh   -> 4x7x7
  conv2 3x3 s2 (4->8)  + hardtanh   -> 8x3x3
  conv3 3x3 s2 (8->16) + hardtanh   -> 16x1x1
  fc1 16->8 + hardtanh, fc2 8->1
with sign() binarization of inputs and weights at every layer.

Lowering:
  * Every post-conv value is a small integer; hardtanh+binarize == sign(),
    and sign(int) == clamp(v,-1,1) (one fused min/max tensor_scalar).
  * Convs become dense matmuls against block-Toeplitz expanded weights
    (225x196, 196x72, 72x16), batch on the matmul free dim.
  * Input sign: (x>=0)-0.5 in {-.5,+.5} (one DVE op per K-chunk), with the
    2x folded into the conv1 weight rows.
  * Tail layers (conv3/fc1/fc2) run as 4 concurrent 32-col/row tiled
    matmuls (tile_position) so their sign ops are [128,128] not [32,512].
Data parallel across 8 NeuronCores, batch split evenly.
"""

import numpy as np
import ml_dtypes
from contextlib import ExitStack

import concourse.bass as bass
import concourse.tile as tile
import concourse.mybir as mybir
from concourse.vector_clock import ScopedClock
from concourse.bass_utils import run_bass_kernel_spmd

# ----------------------------------------------------------------------------
# Workaround: this walrus build accepts only one sync-wait command per
# instruction; Tile attaches several. Split extras onto same-engine NoOps.
# ----------------------------------------------------------------------------
_MAX_WAITS = 1


def _patched_drain_and_barrier(self, tick_clock, wait_clock):
    probe = self.nc.sync.nop()
    wait_clock.add_sem_waits(probe.ins, ScopedClock({None: tick_clock.global_clock}))
    waits = list(probe.ins.sync_info.on_wait)
    probe.ins.sync_info.on_wait = waits[:_MAX_WAITS]
    rest = waits[_MAX_WAITS:]
    while rest:
        nop = self.nc.sync.nop()
        nop.ins.sync_info = mybir.SyncInfo(on_wait=rest[:_MAX_WAITS], on_update=[])
        rest = rest[_MAX_WAITS:]
    self.nc.sync.drain()
    self.nc.all_engine_barrier(sem_only=True)
    assert self.sems is not None
    popped = self.nc._tile_sem_poison_stack.pop()
    assert popped is self._sem_poison
    self.nc.clear_and_free_semaphores(list(self.sems.allocated().values()))
    self.nc.all_engine_barrier(sem_only=True)


tile.TileContext._drain_and_barrier = _patched_drain_and_barrier


def _split_multi_waits(nc, max_waits=_MAX_WAITS):
    """Move extra semaphore waits onto preceding same-engine NoOps."""
    for fn in nc.m.functions:
        for blk in fn.blocks:
            old = list(blk.instructions)
            if not any(
                i.sync_info is not None and len(i.sync_info.on_wait) > max_waits
                for i in old
            ):
                continue
            new = []
            for ins in old:
                si = ins.sync_info
                if si is not None and len(si.on_wait) > max_waits:
                    waits = list(si.on_wait)
                    head, keep = waits[:-max_waits], waits[-max_waits:]
                    for i in range(0, len(head), max_waits):
                        new.append(mybir.InstNoOp(
                            name=f"{ins.name}-sw{i}",
                            engine=ins.engine,
                            bass_nofuse=True,
                            sync_info=mybir.SyncInfo(
                                on_wait=head[i:i + max_waits], on_update=[]
                            ),
                        ))
                    ins.sync_info = mybir.SyncInfo(
                        on_wait=keep, on_update=list(si.on_update)
                    )
                new.append(ins)
            try:
                blk.instructions[:] = new
            except TypeError:
                blk.instructions = new

# ----------------------------------------------------------------------------
# Problem constants
# ----------------------------------------------------------------------------
N_CORES = 8
B_TOTAL = 131072
BC = B_TOTAL // N_CORES  # 16384 samples per core
NB = 512                 # samples per compute tile (one PSUM bank)
GROUP = 2048             # samples per input DMA transfer
K1 = 225                 # conv1 contraction (15*15)
KPAD = 256               # x padded to 256 rows so both DMA chunks are 128-partition
M1 = 196                 # conv1 outputs (4ch * 49pos)
K1A, K1B = 128, 97       # conv1 K split
M1A, M1B = 128, 68       # conv1 M split
K2A, K2B = 128, 68       # conv2 K split (196 total)
M2 = 72                  # conv2 outputs (8ch * 9pos)
BF16 = mybir.dt.bfloat16
F32 = mybir.dt.float32

# engine per elementwise op: 'v'=DVE, 'a'=ACT
ENG = {
    "z1a": "v", "z1b": "a",
    "z2": "a", "z3": "a",
    "z4": "v", "z5": "v",
}

FP8 = mybir.dt.float8e4
M1P = 112                # DR-padded conv1 output half (98 -> 112, step%16==0)
M2P = 80                 # DR-padded conv2 output (72 -> 80)


def _sign_host(w):
    return np.sign(w.astype(np.float32))


def build_weights(w1, w2, w3, wfc1, wfc2):
    """Expand the conv weights into the dense matmul operands."""
    s1, s2, s3 = _sign_host(w1), _sign_host(w2), _sign_host(w3)
    sf1, sf2 = _sign_host(wfc1), _sign_host(wfc2)

    # conv1: [225, 196], x2 to absorb the {-.5,.5} input encoding
    big1 = np.zeros((K1, M1), np.float32)
    for ch in range(4):
        for i in range(7):
            for j in range(7):
                m = ch * 49 + i * 7 + j
                for di in range(3):
                    for dj in range(3):
                        big1[(2 * i + di) * 15 + (2 * j + dj), m] = s1[ch, 0, di, dj]
    big1 *= 2.0

    # conv2: [196, 72]
    big2 = np.zeros((196, M2), np.float32)
    for ch in range(8):
        for i in range(3):
            for j in range(3):
                m = ch * 9 + i * 3 + j
                for ci in range(4):
                    for di in range(3):
                        for dj in range(3):
                            big2[ci * 49 + (2 * i + di) * 7 + (2 * j + dj), m] = \
                                s2[ch, ci, di, dj]

    # conv3: [72, 32]; cols 16:32 zero so the PSUM pad rows are written as 0
    big3 = np.zeros((72, 32), np.float32)
    for ch in range(16):
        for ci in range(8):
            for r in range(3):
                for c in range(3):
                    big3[ci * 9 + r * 3 + c, ch] = s3[ch, ci, r, c]

    # fc1: [32, 32] replicated to [128, 32] for the 4 row/col-tiled matmuls
    f1 = np.zeros((32, 32), np.float32)
    f1[:16, :8] = sf1.T
    f1r = np.tile(f1, (4, 1))
    # fc2: [32, 32] col 0 only, replicated to [128, 32]
    f2 = np.zeros((32, 32), np.float32)
    f2[:8, 0] = sf2[0, :]
    f2r = np.tile(f2, (4, 1))

    bf = ml_dtypes.bfloat16
    return {
        "w1": big1.astype(bf), "w2": big2.astype(bf), "w3": big3.astype(bf),
        "wf1": f1r.astype(bf), "wf2": f2r.astype(bf),
    }


def _sign_int(nc, eng, out_ap, in_ap):
    """sign() for integer-valued input: clamp(v,-1,1) on DVE, LUT on ACT."""
    if eng == "a":
        nc.scalar.sign(out_ap, in_ap)
    else:
        nc.vector.tensor_scalar(out_ap, in_ap, 1.0, -1.0,
                                mybir.AluOpType.min, mybir.AluOpType.max)


def _sign_x(nc, eng, out_ap, in_ap):
    """(x>=0) - 0.5 -> {-0.5, +0.5}; the 2x lives in the conv1 weights."""
    assert eng == "v"
    nc.vector.tensor_scalar(out_ap, in_ap, 0.0, 0.5,
                            mybir.AluOpType.is_ge, mybir.AluOpType.subtract)


def build_bass(bc=BC, split_waits=True):
    """Build the per-core Bass program (identical on all cores)."""
    assert bc % GROUP == 0
    n_groups = bc // GROUP
    tiles_per_group = GROUP // NB

    nc = bass.Bass()
    xt = nc.dram_tensor("xt", (KPAD, bc), F32, kind="ExternalInput")
    w1 = nc.dram_tensor("w1", (K1, M1), BF16, kind="ExternalInput")
    w2 = nc.dram_tensor("w2", (196, M2), BF16, kind="ExternalInput")
    w3 = nc.dram_tensor("w3", (72, 32), BF16, kind="ExternalInput")
    wf1 = nc.dram_tensor("wf1", (128, 32), BF16, kind="ExternalInput")
    wf2 = nc.dram_tensor("wf2", (128, 32), BF16, kind="ExternalInput")
    y = nc.dram_tensor("y", (4, bc // 4), F32, kind="ExternalOutput")

    with tile.TileContext(nc) as tc:
        with ExitStack() as ctx:
            consts = ctx.enter_context(tc.tile_pool(name="consts", bufs=1))
            xin = ctx.enter_context(tc.tile_pool(name="xin", bufs=2))
            sx_p = ctx.enter_context(tc.tile_pool(name="sx", bufs=2))
            s1_p = ctx.enter_context(tc.tile_pool(name="s1", bufs=3))
            s2_p = ctx.enter_context(tc.tile_pool(name="s2", bufs=3))
            s3_p = ctx.enter_context(tc.tile_pool(name="s3", bufs=3))
            s4_p = ctx.enter_context(tc.tile_pool(name="s4", bufs=3))
            yout = ctx.enter_context(tc.tile_pool(name="yout", bufs=2))
            ps_big = ctx.enter_context(tc.tile_pool(name="ps_big", bufs=2, space="PSUM"))
            ps_mid = ctx.enter_context(tc.tile_pool(name="ps_mid", bufs=2, space="PSUM"))
            ps_tail = ctx.enter_context(tc.tile_pool(name="ps_tail", bufs=2, space="PSUM"))

            # --- load weights once ---
            w1hi = consts.tile([K1A, M1], BF16)
            w1lo = consts.tile([K1B, M1], BF16)
            nc.sync.dma_start(out=w1hi, in_=w1[0:K1A, :])
            nc.sync.dma_start(out=w1lo, in_=w1[K1A:K1, :])
            w2hi = consts.tile([K2A, M2], BF16)
            w2lo = consts.tile([K2B, M2], BF16)
            nc.sync.dma_start(out=w2hi, in_=w2[0:K2A, :])
            nc.sync.dma_start(out=w2lo, in_=w2[K2A:196, :])
            w3t = consts.tile([72, 32], BF16)
            nc.sync.dma_start(out=w3t, in_=w3[:, :])
            wf1t = consts.tile([128, 32], BF16)
            nc.sync.dma_start(out=wf1t, in_=wf1[:, :])
            wf2t = consts.tile([128, 32], BF16)
            nc.sync.dma_start(out=wf2t, in_=wf2[:, :])

            for g in range(n_groups):
                g0 = g * GROUP
                # SWDGE DMA casts fp32 -> bf16 on the fly. The first group is
                # loaded and signed per-tile to prime the pipeline quickly.
                xa = xin.tile([128, GROUP], F32, tag="xa")
                xb = xin.tile([128, GROUP], F32, tag="xb")
                n_chunks = tiles_per_group if g == 0 else 1
                cw = GROUP // n_chunks
                for ci in range(n_chunks):
                    cs = slice(ci * cw, (ci + 1) * cw)
                    gs = slice(g0 + ci * cw, g0 + (ci + 1) * cw)
                    nc.sync.dma_start(out=xa[:, cs], in_=xt[0:128, gs])
                    nc.sync.dma_start(out=xb[:, cs], in_=xt[128:256, gs])
                sxs = []
                for ti in range(tiles_per_group):
                    ts_ = slice(ti * NB, (ti + 1) * NB)
                    sxa = sx_p.tile([K1A, NB], BF16, name=f"sxa_{ti}", tag=f"sxa{ti % 2}")
                    sxb = sx_p.tile([K1B, NB], BF16, name=f"sxb_{ti}", tag=f"sxb{ti % 2}")
                    _sign_x(nc, "v", sxa, xa[:, ts_])
                    _sign_x(nc, "v", sxb, xb[0:K1B, ts_])
                    sxs.append((sxa, sxb))
                ysb = yout.tile([128, GROUP // 4], F32, tag="ysb")

                # process tiles in pairs so each big LDWEIGHTS serves 2 matmuls
                for tp in range(tiles_per_group // 2):
                    sls = [slice((2 * tp + uu) * NB, (2 * tp + uu + 1) * NB)
                           for uu in range(2)]

                    p1as = [ps_big.tile([M1A, NB], F32, name=f"p1a_{u}", tag="p1a")
                            for u in range(2)]
                    p1bs = [ps_big.tile([M1B, NB], F32, name=f"p1b_{u}", tag="p1b")
                            for u in range(2)]
                    for u in range(2):
                        nc.tensor.matmul(p1as[u], w1hi[:, 0:M1A], sxs[2 * tp + u][0],
                                         start=True, stop=False)
                    for u in range(2):
                        nc.tensor.matmul(p1as[u], w1lo[:, 0:M1A], sxs[2 * tp + u][1],
                                         start=False, stop=True)
                    for u in range(2):
                        nc.tensor.matmul(p1bs[u], w1hi[:, M1A:M1], sxs[2 * tp + u][0],
                                         start=True, stop=False)
                    for u in range(2):
                        nc.tensor.matmul(p1bs[u], w1lo[:, M1A:M1], sxs[2 * tp + u][1],
                                         start=False, stop=True)

                    s1as, s1bs = [], []
                    for u in range(2):
                        s1a = s1_p.tile([M1A, NB], BF16, name=f"s1a_{u}", tag=f"s1a{u}")
                        s1b = s1_p.tile([M1B, NB], BF16, name=f"s1b_{u}", tag=f"s1b{u}")
                        _sign_int(nc, ENG["z1a"], s1a, p1as[u])
                        _sign_int(nc, ENG["z1b"], s1b, p1bs[u])
                        s1as.append(s1a)
                        s1bs.append(s1b)

                    p2s = [ps_mid.tile([M2, NB], F32, name=f"p2_{u}", tag="p2")
                           for u in range(2)]
                    for u in range(2):
                        nc.tensor.matmul(p2s[u], w2hi, s1as[u], start=True, stop=False)
                    for u in range(2):
                        nc.tensor.matmul(p2s[u], w2lo, s1bs[u], start=False, stop=True)

                    for u in range(2):
                        t = 2 * tp + u
                        s2t = s2_p.tile([M2, NB], BF16, tag="s2")
                        _sign_int(nc, ENG["z2"], s2t, p2s[u])

                        # --- tail: 4 concurrent 32-wide tiled matmuls each;
                        # all 3 stages share one PSUM bank ---
                        ptail = ps_tail.tile([128, 384], F32, tag="tail")
                        p3 = ptail[:, 0:128]
                        for r in range(4):
                            nc.tensor.matmul(
                                p3[32 * r:32 * r + 32, :], w3t,
                                s2t[:, 128 * r:128 * r + 128],
                                start=True, stop=True, tile_position=(0, 32 * r),
                            )
                        s3t = s3_p.tile([128, 128], BF16, tag="s3")
                        _sign_int(nc, ENG["z3"], s3t, p3)

                        p4 = ptail[:, 128:256]
                        for r in range(4):
                            nc.tensor.matmul(
                                p4[32 * r:32 * r + 32, :],
                                wf1t[32 * r:32 * r + 32, :],
                                s3t[32 * r:32 * r + 32, :],
                                start=True, stop=True, tile_position=(32 * r, 32 * r),
                            )
                        s4t = s4_p.tile([128, 128], BF16, tag="s4")
                        _sign_int(nc, ENG["z4"], s4t, p4)

                        p5 = ptail[:, 256:384]
                        for r in range(4):
                            nc.tensor.matmul(
                                p5[32 * r:32 * r + 32, :],
                                wf2t[32 * r:32 * r + 32, :],
                                s4t[32 * r:32 * r + 32, :],
                                start=True, stop=True, tile_position=(32 * r, 32 * r),
                            )
                        nc.vector.tensor_copy(ysb[:, t * 128:(t + 1) * 128], p5)

                # y rows live at partitions {0,32,64,96}
                nc.sync.dma_start(
                    out=y[0:4, g * (GROUP // 4):(g + 1) * (GROUP // 4)],
                    in_=ysb[0:128:32, :],
                )
    if split_waits:
        _split_multi_waits(nc)
    return nc


_NC_CACHE = {}


def _get_nc(bc):
    if bc not in _NC_CACHE:
        _NC_CACHE[bc] = build_bass(bc)
    return _NC_CACHE[bc]


def host_inputs(x, w1, w2, w3, wfc1, wfc2, n_cores=N_CORES):
    """Shard + lay out the inputs for each core.

    xt layout: [128 pairs, 2, bc] where xt[k, i, s] = x[s, feature 2k+i]
    (features 225..255 zero-padded) — the DoubleRow rhs layout.
    """
    b = x.shape[0]
    bc = b // n_cores
    ws = build_weights(w1, w2, w3, wfc1, wfc2)
    xf = np.asarray(x, np.float32).reshape(b, K1)
    in_maps = []
    for c in range(n_cores):
        xc = np.zeros((KPAD, bc), np.float32)
        xc[:K1] = xf[c * bc:(c + 1) * bc].T
        in_maps.append({"xt": xc, **ws})
    return in_maps, bc


def unshard_y(results, bc):
    """[4, bc/4] per core -> [8*bc, 1]; sample = g*2048 + t*512 + r*128 + c."""
    outs = []
    for r in results:
        yb = np.asarray(r["y"], np.float32)          # [4, bc/4]
        n_groups = bc // GROUP
        a = yb.reshape(4, n_groups, 4, 128)           # [r, g, t, c]
        outs.append(a.transpose(1, 2, 0, 3).reshape(bc))
    return np.concatenate(outs).reshape(-1, 1)


def kernel(x, w1, w2, w3, wfc1, wfc2):
    in_maps, bc = host_inputs(x, w1, w2, w3, wfc1, wfc2)
    nc = _get_nc(bc)
    res = run_bass_kernel_spmd(nc, in_maps, core_ids=list(range(N_CORES)))
    return unshard_y(res.results, bc).astype(np.float32)


# revision 25
# speedup vs baseline: 1.2552x; 1.0549x over previous
"""Trainium2 Bass kernel for a binarized CNN (BinaryNet-style).

Network (per sample, input 1x15x15):
  conv1 3x3 s2 (1->4)  + hardtanh   -> 4x7x7
  conv2 3x3 s2 (4->8)  + hardtanh   -> 8x3x3
  conv3 3x3 s2 (8->16) + hardtanh   -> 16x1x1
  fc1 16->8 + hardtanh, fc2 8->1
with sign() binarization of inputs and weights at every layer.

Lowering:
  * Every post-conv value is a small integer; hardtanh+binarize == sign(),
    and sign(int) == clamp(v,-1,1) (one fused min/max tensor_scalar).
  * Convs become dense matmuls against block-Toeplitz expanded weights
    (225x196, 196x72, 72x16), batch on the matmul free dim.
  * Input sign: (x>=0)-0.5 in {-.5,+.5} (one DVE op per K-chunk), with the
    2x folded into the conv1 weight rows.
  * Tail layers (conv3/fc1/fc2) run as 4 concurrent 32-col/row tiled
    matmuls (tile_position) so their sign ops are [128,128] not [32,512].
Data parallel across 8 NeuronCores, batch split evenly.
"""

import numpy as np
import ml_dtypes
from contextlib import ExitStack

import concourse.bass as bass
import concourse.tile as tile
import concourse.mybir as mybir
from concourse.vector_clock import ScopedClock
from concourse.bass_utils import run_bass_kernel_spmd

# ----------------------------------------------------------------------------
# Workaround: this walrus build accepts only one sync-wait command per
# instruction; Tile attaches several. Split extras onto same-engine NoOps.
# ----------------------------------------------------------------------------
_MAX_WAITS = 1


def _patched_drain_and_barrier(self, tick_clock, wait_clock):
    probe = self.nc.sync.nop()
    wait_clock.add_sem_waits(probe.ins, ScopedClock({None: tick_clock.global_clock}))
    waits = list(probe.ins.sync_info.on_wait)
    probe.ins.sync_info.on_wait = waits[:_MAX_WAITS]
    rest = waits[_MAX_WAITS:]
    while rest:
        nop = self.nc.sync.nop()
        nop.ins.sync_info = mybir.SyncInfo(on_wait=rest[:_MAX_WAITS], on_update=[])
        rest = rest[_MAX_WAITS:]
    self.nc.sync.drain()
    self.nc.all_engine_barrier(sem_only=True)
    assert self.sems is not None
    popped = self.nc._tile_sem_poison_stack.pop()
    assert popped is self._sem_poison
    self.nc.clear_and_free_semaphores(list(self.sems.allocated().values()))
    self.nc.all_engine_barrier(sem_only=True)


tile.TileContext._drain_and_barrier = _patched_drain_and_barrier


def _split_multi_waits(nc, max_waits=_MAX_WAITS):
    """Move extra semaphore waits onto preceding same-engine NoOps."""
    for fn in nc.m.functions:
        for blk in fn.blocks:
            old = list(blk.instructions)
            if not any(
                i.sync_info is not None and len(i.sync_info.on_wait) > max_waits
                for i in old
            ):
                continue
            new = []
            for ins in old:
                si = ins.sync_info
                if si is not None and len(si.on_wait) > max_waits:
                    waits = list(si.on_wait)
                    head, keep = waits[:-max_waits], waits[-max_waits:]
                    for i in range(0, len(head), max_waits):
                        new.append(mybir.InstNoOp(
                            name=f"{ins.name}-sw{i}",
                            engine=ins.engine,
                            bass_nofuse=True,
                            sync_info=mybir.SyncInfo(
                                on_wait=head[i:i + max_waits], on_update=[]
                            ),
                        ))
                    ins.sync_info = mybir.SyncInfo(
                        on_wait=keep, on_update=list(si.on_update)
                    )
                new.append(ins)
            try:
                blk.instructions[:] = new
            except TypeError:
                blk.instructions = new

# ----------------------------------------------------------------------------
# Problem constants
# ----------------------------------------------------------------------------
N_CORES = 8
B_TOTAL = 131072
BC = B_TOTAL // N_CORES  # 16384 samples per core
NB = 512                 # samples per compute tile (one PSUM bank)
GROUP = 2048             # samples per input DMA transfer
K1 = 225                 # conv1 contraction (15*15)
KPAD = 256               # x padded to 256 rows so both DMA chunks are 128-partition
M1 = 196                 # conv1 outputs (4ch * 49pos)
K1A, K1B = 128, 97       # conv1 K split
M1A, M1B = 128, 68       # conv1 M split
K2A, K2B = 128, 68       # conv2 K split (196 total)
M2 = 72                  # conv2 outputs (8ch * 9pos)
BF16 = mybir.dt.bfloat16
F32 = mybir.dt.float32

# engine per elementwise op: 'v'=DVE, 'a'=ACT
ENG = {
    "z1a": "v", "z1b": "a",
    "z2": "a", "z3": "a",
    "z4": "v", "z5": "v",
}

FP8 = mybir.dt.float8e4
M1P = 112                # DR-padded conv1 output half (98 -> 112, step%16==0)
M2P = 80                 # DR-padded conv2 output (72 -> 80)


def _sign_host(w):
    return np.sign(w.astype(np.float32))


def build_weights(w1, w2, w3, wfc1, wfc2):
    """Expand the conv weights into the dense matmul operands."""
    s1, s2, s3 = _sign_host(w1), _sign_host(w2), _sign_host(w3)
    sf1, sf2 = _sign_host(wfc1), _sign_host(wfc2)

    # conv1: [225, 196], x2 to absorb the {-.5,.5} input encoding
    big1 = np.zeros((K1, M1), np.float32)
    for ch in range(4):
        for i in range(7):
            for j in range(7):
                m = ch * 49 + i * 7 + j
                for di in range(3):
                    for dj in range(3):
                        big1[(2 * i + di) * 15 + (2 * j + dj), m] = s1[ch, 0, di, dj]
    big1 *= 2.0

    # conv2: [196, 72]
    big2 = np.zeros((196, M2), np.float32)
    for ch in range(8):
        for i in range(3):
            for j in range(3):
                m = ch * 9 + i * 3 + j
                for ci in range(4):
                    for di in range(3):
                        for dj in range(3):
                            big2[ci * 49 + (2 * i + di) * 7 + (2 * j + dj), m] = \
                                s2[ch, ci, di, dj]

    # conv3: [72, 32]; cols 16:32 zero so the PSUM pad rows are written as 0
    big3 = np.zeros((72, 32), np.float32)
    for ch in range(16):
        for ci in range(8):
            for r in range(3):
                for c in range(3):
                    big3[ci * 9 + r * 3 + c, ch] = s3[ch, ci, r, c]

    # fc1: [32, 32] replicated to [128, 32] for the 4 row/col-tiled matmuls
    f1 = np.zeros((32, 32), np.float32)
    f1[:16, :8] = sf1.T
    f1r = np.tile(f1, (4, 1))
    # fc2: [32, 32] col 0 only, replicated to [128, 32]
    f2 = np.zeros((32, 32), np.float32)
    f2[:8, 0] = sf2[0, :]
    f2r = np.tile(f2, (4, 1))

    bf = ml_dtypes.bfloat16
    return {
        "w1": big1.astype(bf), "w2": big2.astype(bf), "w3": big3.astype(bf),
        "wf1": f1r.astype(bf), "wf2": f2r.astype(bf),
    }


def _sign_int(nc, eng, out_ap, in_ap):
    """sign() for integer-valued input: clamp(v,-1,1) on DVE, LUT on ACT."""
    if eng == "a":
        nc.scalar.sign(out_ap, in_ap)
    else:
        nc.vector.tensor_scalar(out_ap, in_ap, 1.0, -1.0,
                                mybir.AluOpType.min, mybir.AluOpType.max)


def _sign_x(nc, eng, out_ap, in_ap):
    """(x>=0) - 0.5 -> {-0.5, +0.5}; the 2x lives in the conv1 weights."""
    assert eng == "v"
    nc.vector.tensor_scalar(out_ap, in_ap, 0.0, 0.5,
                            mybir.AluOpType.is_ge, mybir.AluOpType.subtract)


def build_bass(bc=BC, split_waits=True):
    """Build the per-core Bass program (identical on all cores)."""
    assert bc % GROUP == 0
    n_groups = bc // GROUP
    tiles_per_group = GROUP // NB

    nc = bass.Bass()
    xt = nc.dram_tensor("xt", (KPAD, bc), F32, kind="ExternalInput")
    w1 = nc.dram_tensor("w1", (K1, M1), BF16, kind="ExternalInput")
    w2 = nc.dram_tensor("w2", (196, M2), BF16, kind="ExternalInput")
    w3 = nc.dram_tensor("w3", (72, 32), BF16, kind="ExternalInput")
    wf1 = nc.dram_tensor("wf1", (128, 32), BF16, kind="ExternalInput")
    wf2 = nc.dram_tensor("wf2", (128, 32), BF16, kind="ExternalInput")
    y = nc.dram_tensor("y", (4, bc // 4), F32, kind="ExternalOutput")

    with tile.TileContext(nc) as tc:
        with ExitStack() as ctx:
            consts = ctx.enter_context(tc.tile_pool(name="consts", bufs=1))
            xin = ctx.enter_context(tc.tile_pool(name="xin", bufs=2))
            sx_p = ctx.enter_context(tc.tile_pool(name="sx", bufs=2))
            s1_p = ctx.enter_context(tc.tile_pool(name="s1", bufs=3))
            s2_p = ctx.enter_context(tc.tile_pool(name="s2", bufs=3))
            s3_p = ctx.enter_context(tc.tile_pool(name="s3", bufs=3))
            s4_p = ctx.enter_context(tc.tile_pool(name="s4", bufs=3))
            yout = ctx.enter_context(tc.tile_pool(name="yout", bufs=2))
            ps_big = ctx.enter_context(tc.tile_pool(name="ps_big", bufs=2, space="PSUM"))
            ps_mid = ctx.enter_context(tc.tile_pool(name="ps_mid", bufs=2, space="PSUM"))
            ps_tail = ctx.enter_context(tc.tile_pool(name="ps_tail", bufs=1, space="PSUM"))

            # --- load weights once ---
            w1hi = consts.tile([K1A, M1], BF16)
            w1lo = consts.tile([K1B, M1], BF16)
            nc.sync.dma_start(out=w1hi, in_=w1[0:K1A, :])
            nc.sync.dma_start(out=w1lo, in_=w1[K1A:K1, :])
            w2hi = consts.tile([K2A, M2], BF16)
            w2lo = consts.tile([K2B, M2], BF16)
            nc.sync.dma_start(out=w2hi, in_=w2[0:K2A, :])
            nc.sync.dma_start(out=w2lo, in_=w2[K2A:196, :])
            w3t = consts.tile([72, 32], BF16)
            nc.sync.dma_start(out=w3t, in_=w3[:, :])
            wf1t = consts.tile([128, 32], BF16)
            nc.sync.dma_start(out=wf1t, in_=wf1[:, :])
            wf2t = consts.tile([128, 32], BF16)
            nc.sync.dma_start(out=wf2t, in_=wf2[:, :])

            for g in range(n_groups):
                g0 = g * GROUP
                # SWDGE DMA casts fp32 -> bf16 on the fly. The first group is
                # loaded and signed per-tile to prime the pipeline quickly.
                xa = xin.tile([128, GROUP], F32, tag="xa")
                xb = xin.tile([128, GROUP], F32, tag="xb")
                n_chunks = tiles_per_group if g == 0 else 1
                cw = GROUP // n_chunks
                for ci in range(n_chunks):
                    cs = slice(ci * cw, (ci + 1) * cw)
                    gs = slice(g0 + ci * cw, g0 + (ci + 1) * cw)
                    nc.sync.dma_start(out=xa[:, cs], in_=xt[0:128, gs])
                    nc.sync.dma_start(out=xb[:, cs], in_=xt[128:256, gs])
                sxs = []
                for ti in range(tiles_per_group):
                    ts_ = slice(ti * NB, (ti + 1) * NB)
                    sxa = sx_p.tile([K1A, NB], BF16, name=f"sxa_{ti}", tag=f"sxa{ti % 2}")
                    sxb = sx_p.tile([K1B, NB], BF16, name=f"sxb_{ti}", tag=f"sxb{ti % 2}")
                    _sign_x(nc, "v", sxa, xa[:, ts_])
                    _sign_x(nc, "v", sxb, xb[0:K1B, ts_])
                    sxs.append((sxa, sxb))
                ysb = yout.tile([128, GROUP // 4], F32, tag="ysb")

                # process tiles in pairs so each big LDWEIGHTS serves 2 matmuls
                for tp in range(tiles_per_group // 2):
                    sls = [slice((2 * tp + uu) * NB, (2 * tp + uu + 1) * NB)
                           for uu in range(2)]

                    p1as = [ps_big.tile([M1A, NB], F32, name=f"p1a_{u}", tag="p1a")
                            for u in range(2)]
                    p1bs = [ps_big.tile([M1B, NB], F32, name=f"p1b_{u}", tag="p1b")
                            for u in range(2)]
                    for u in range(2):
                        nc.tensor.matmul(p1as[u], w1hi[:, 0:M1A], sxs[2 * tp + u][0],
                                         start=True, stop=False)
                    for u in range(2):
                        nc.tensor.matmul(p1as[u], w1lo[:, 0:M1A], sxs[2 * tp + u][1],
                                         start=False, stop=True)
                    for u in range(2):
                        nc.tensor.matmul(p1bs[u], w1hi[:, M1A:M1], sxs[2 * tp + u][0],
                                         start=True, stop=False)
                    for u in range(2):
                        nc.tensor.matmul(p1bs[u], w1lo[:, M1A:M1], sxs[2 * tp + u][1],
                                         start=False, stop=True)

                    s1as, s1bs = [], []
                    for u in range(2):
                        s1a = s1_p.tile([M1A, NB], BF16, name=f"s1a_{u}", tag=f"s1a{u}")
                        s1b = s1_p.tile([M1B, NB], BF16, name=f"s1b_{u}", tag=f"s1b{u}")
                        _sign_int(nc, ENG["z1a"], s1a, p1as[u])
                        _sign_int(nc, ENG["z1b"], s1b, p1bs[u])
                        s1as.append(s1a)
                        s1bs.append(s1b)

                    p2s = [ps_mid.tile([M2, NB], F32, name=f"p2_{u}", tag="p2")
                           for u in range(2)]
                    for u in range(2):
                        nc.tensor.matmul(p2s[u], w2hi, s1as[u], start=True, stop=False)
                    for u in range(2):
                        nc.tensor.matmul(p2s[u], w2lo, s1bs[u], start=False, stop=True)

                    s2s = []
                    for u in range(2):
                        s2t = s2_p.tile([M2, NB], BF16, name=f"s2_{u}", tag=f"s2{u}")
                        _sign_int(nc, ENG["z2"], s2t, p2s[u])
                        s2s.append(s2t)

                    # --- tail for the pair: 8 concurrent 32-wide tiled
                    # matmuls per stage (N=256 each), one 2-bank PSUM tile ---
                    NT = 256
                    ptail = ps_tail.tile([128, 3, NT], F32, tag="tail")
                    p3 = ptail[:, 0, :]
                    for u in range(2):
                        for r in range(4):
                            nc.tensor.matmul(
                                p3[32 * r:32 * r + 32, u * 128:u * 128 + 128],
                                w3t, s2s[u][:, 128 * r:128 * r + 128],
                                start=True, stop=True, tile_position=(0, 32 * r),
                            )
                    s3t = s3_p.tile([128, NT], BF16, tag="s3")
                    _sign_int(nc, ENG["z3"], s3t, p3)

                    p4 = ptail[:, 1, :]
                    for r in range(4):
                        nc.tensor.matmul(
                            p4[32 * r:32 * r + 32, :],
                            wf1t[32 * r:32 * r + 32, :],
                            s3t[32 * r:32 * r + 32, :],
                            start=True, stop=True, tile_position=(32 * r, 32 * r),
                        )
                    s4t = s4_p.tile([128, NT], BF16, tag="s4")
                    _sign_int(nc, ENG["z4"], s4t, p4)

                    p5 = ptail[:, 2, :]
                    for r in range(4):
                        nc.tensor.matmul(
                            p5[32 * r:32 * r + 32, :],
                            wf2t[32 * r:32 * r + 32, :],
                            s4t[32 * r:32 * r + 32, :],
                            start=True, stop=True, tile_position=(32 * r, 32 * r),
                        )
                    nc.vector.tensor_copy(ysb[:, tp * NT:(tp + 1) * NT], p5)

                # y rows live at partitions {0,32,64,96}
                nc.sync.dma_start(
                    out=y[0:4, g * (GROUP // 4):(g + 1) * (GROUP // 4)],
                    in_=ysb[0:128:32, :],
                )
    if split_waits:
        _split_multi_waits(nc)
    return nc


_NC_CACHE = {}


def _get_nc(bc):
    if bc not in _NC_CACHE:
        _NC_CACHE[bc] = build_bass(bc)
    return _NC_CACHE[bc]


def host_inputs(x, w1, w2, w3, wfc1, wfc2, n_cores=N_CORES):
    """Shard + lay out the inputs for each core.

    xt layout: [128 pairs, 2, bc] where xt[k, i, s] = x[s, feature 2k+i]
    (features 225..255 zero-padded) — the DoubleRow rhs layout.
    """
    b = x.shape[0]
    bc = b // n_cores
    ws = build_weights(w1, w2, w3, wfc1, wfc2)
    xf = np.asarray(x, np.float32).reshape(b, K1)
    in_maps = []
    for c in range(n_cores):
        xc = np.zeros((KPAD, bc), np.float32)
        xc[:K1] = xf[c * bc:(c + 1) * bc].T
        in_maps.append({"xt": xc, **ws})
    return in_maps, bc


def unshard_y(results, bc):
    """[4, bc/4] per core -> [8*bc, 1]; sample = g*2048 + t*512 + r*128 + c."""
    outs = []
    for r in results:
        yb = np.asarray(r["y"], np.float32)          # [4, bc/4]
        n_groups = bc // GROUP
        a = yb.reshape(4, n_groups, 4, 128)           # [r, g, t, c]
        outs.append(a.transpose(1, 2, 0, 3).reshape(bc))
    return np.concatenate(outs).reshape(-1, 1)


def kernel(x, w1, w2, w3, wfc1, wfc2):
    in_maps, bc = host_inputs(x, w1, w2, w3, wfc1, wfc2)
    nc = _get_nc(bc)
    res = run_bass_kernel_spmd(nc, in_maps, core_ids=list(range(N_CORES)))
    return unshard_y(res.results, bc).astype(np.float32)
